# revision 3
# baseline (speedup 1.0000x reference)
"""nn_Attention_21285857919430: GroupNorm + single-head attention block.

Strategy: data-parallel over batch across the 8 NeuronCores (2 samples per
core); the small (C,C) weights are replicated to every core. Math per sample
is identical to the reference (including the faithful non-transposing
reshape of the (b*n, c) projection buffers back to (b, c, n)).

All shapes are hardcoded per the problem spec:
  x: (16, 512, 64, 64) f32, weights (512,512), biases (512,)
"""

import numpy as np
import jax
import jax.numpy as jnp

B, C, H, W = 16, 512, 64, 64
G = 32
EPS = 1e-5
N_CORES = 8

_WEIGHT_KEYS = ["gn_w", "gn_b", "Wq", "bq", "Wk", "bk", "Wv", "bv", "Wo", "bo"]


def _forward(x, gn_w, gn_b, Wq, bq, Wk, bk, Wv, bv, Wo, bo):
    # x: (local_b, C, H, W) — per-core shard of the batch.
    #
    # Transpose-light formulation. The reference computes qf = xnf @ Wq.T on
    # the (b*n, c) transposed activations, then REINTERPRETS that buffer as
    # (b, c, n) row-major. Algebraically, with qfT = Wq @ xn + bq (shape
    # (c, n), no big transpose needed) and n split as (j1=8, j0=512):
    #   q(c-major)[i, j1*512+j0] = qfT[j0, i*8+j1]
    # so scores/att become multi-dim dot_general contractions over the
    # reshaped qfT/kfT/vfT buffers — XLA contracts them directly instead of
    # materializing (b,n,c) transposes through DVE transpose kernels. Only
    # the single final ofT -> of transpose is materialized.
    b, c, h, w = x.shape
    n = h * w
    x3 = x.reshape(b, c, n)
    xg = x3.reshape(b, G, (c // G) * n)
    mu = xg.mean(-1, keepdims=True)
    var = xg.var(-1, keepdims=True)
    xn = ((xg - mu) / jnp.sqrt(var + EPS)).reshape(b, c, n)
    xn = xn * gn_w[None, :, None] + gn_b[None, :, None]
    # Matmuls run with bf16 operands + fp32 accumulation (4x PE rate vs
    # fp32's 4-cycles-per-row); GroupNorm, softmax, bias adds, and the
    # residual stay fp32.
    bf = jnp.bfloat16
    f32 = jnp.float32
    xnb = xn.astype(bf)
    qfT = jnp.einsum("cd,bdn->bcn", Wq.astype(bf), xnb,
                     preferred_element_type=f32) + bq[None, :, None]
    kfT = jnp.einsum("cd,bdn->bcn", Wk.astype(bf), xnb,
                     preferred_element_type=f32) + bk[None, :, None]
    vfT = jnp.einsum("cd,bdn->bcn", Wv.astype(bf), xnb,
                     preferred_element_type=f32) + bv[None, :, None]
    qfT4 = qfT.reshape(b, c, c, 8)  # (b, j0, i, j1)
    kfT4 = kfT.reshape(b, c, c, 8)
    vfT4 = vfT.reshape(b, c, c, 8)
    scale = 1.0 / jnp.sqrt(jnp.float32(c))
    scores = jnp.einsum("bkcj,bkdj->bcd", qfT4.astype(bf), kfT4.astype(bf),
                        preferred_element_type=f32) * scale
    weights = jax.nn.softmax(scores, axis=-1)
    att = jnp.einsum("bcd,bkdj->bcjk", weights.astype(bf), vfT4.astype(bf),
                     preferred_element_type=f32).reshape(b, c, n)
    ofT = jnp.einsum("ce,ben->bcn", Wo.astype(bf), att.astype(bf),
                     preferred_element_type=f32) + bo[None, :, None]
    out = ofT.transpose(0, 2, 1).reshape(b, c, n)
    return (x3 + out).reshape(b, c, h, w)


_pmapped = jax.pmap(_forward, in_axes=(0,) + (None,) * 10)


def kernel(**inputs) -> np.ndarray:
    x = np.asarray(inputs["x"], dtype=np.float32)
    shard = B // N_CORES  # 2 samples per core
    xs = x.reshape(N_CORES, shard, C, H, W)
    rest = [np.asarray(inputs[k], dtype=np.float32) for k in _WEIGHT_KEYS]
    out = _pmapped(xs, *rest)
    return np.asarray(out).reshape(B, C, H, W).astype(np.float32)


if __name__ == "__main__":
    rng = np.random.default_rng(0)
    demo = {
        "x": rng.standard_normal((B, C, H, W), dtype=np.float32),
        "gn_w": np.ones((C,), np.float32),
        "gn_b": np.zeros((C,), np.float32),
    }
    for nm in ["Wq", "Wk", "Wv", "Wo"]:
        demo[nm] = (rng.standard_normal((C, C)) * 0.02).astype(np.float32)
    for nm in ["bq", "bk", "bv", "bo"]:
        demo[nm] = (rng.standard_normal((C,)) * 0.02).astype(np.float32)
    y = kernel(**demo)
    print("ok", y.shape, y.dtype)
